# revision 1
# baseline (speedup 1.0000x reference)
"""Gated GQA self-attention with KV cache, tensor-parallel over heads on 8
Trainium2 NeuronCores.

Reference computation (fp32):
    q = rms_norm((x @ w_q.T).reshape(B,L,H,HD))      # per-head rms over HD
    k = rms_norm((x @ w_k.T).reshape(B,L,HKV,HD))
    v = (x @ w_v.T).reshape(B,L,HKV,HD)
    k_t/v_t = concat(cache, new) over seq -> [B,HKV,S,HD]
    o = softmax(q @ k_t.T / sqrt(HD)) @ v_t          # full (non-causal)
    o *= sigmoid(x[..., :16] @ w_gate.T)             # per-head gate
    y = o.reshape(B,L,D) @ w_out.T

Sharding: core c owns q heads {2c, 2c+1} and kv group g=c//2 (GQA groups
stay intact).  Each core computes its heads' attention plus the partial
out-projection y_c = o_c @ w_out[:, cols_c].T; the host sums the 8
partials (replaces the all-reduce).

Device-side layout: everything is computed feature-on-partition
("transposed"), so scores come out [s, l] and the P matrix never needs a
transpose for the p@v matmul.  The host pre-transposes x and the weights
so the device never transposes activations either.  Matmuls run in
float32r (fp32 stored with tf32-like operand rounding, ~5e-4 relative;
1 cycle/row when the moving free dim >= 256 vs 4 for plain fp32).

Softmax runs without max-subtraction (scores here are ~N(0,1); exp
cannot overflow).  The softmax denominator is a ones-matmul partition
sum; per-column factors (q-rms scale, gate/denominator) are applied as
rank-1 PE broadcasts; the k-rms scale rides the exp activation's
per-partition scale operand.  1-partition rows are reshaped to [128, n]
via small internal-DRAM bounce DMAs so reciprocals never run on a
single DVE lane.
"""

from contextlib import ExitStack

import numpy as np

import concourse.bass as bass
import concourse.tile as tile
from concourse import bacc, mybir
from concourse.bass_utils import run_bass_kernel_spmd

F32R = mybir.dt.float32r
F32 = mybir.dt.float32
AF = mybir.ActivationFunctionType

B, L, D = 2, 1024, 2048
H, HKV, HD = 16, 4, 128
CACHE = 1024
BL = B * L                  # 2048
S = CACHE + L               # 2048
NCORES = 8
QH = H // NCORES            # 2 q heads per core
JC = QH * HD                # 256 out-proj contraction cols per core
EPS = 1e-6

_CACHED_NC = None


def _build_core_program():
    """One SPMD program; per-core differences are input data only."""
    nc = bacc.Bacc("TRN2", target_bir_lowering=False, debug=False)

    xt = nc.dram_tensor("xt", [D, BL], F32R, kind="ExternalInput").ap()
    wqkv = nc.dram_tensor("wqkv", [D, 4 * HD], F32R, kind="ExternalInput").ap()
    wo = nc.dram_tensor("wo", [JC, D], F32R, kind="ExternalInput").ap()
    wg = nc.dram_tensor("wg", [H, QH], F32R, kind="ExternalInput").ap()
    ckt = nc.dram_tensor("ckt", [B, HD, CACHE], F32R, kind="ExternalInput").ap()
    cv = nc.dram_tensor("cv", [B, CACHE, HD], F32R, kind="ExternalInput").ap()
    # [:, :128] identity for PE transposes, [:, 128] all-ones column
    consts_in = nc.dram_tensor("consts", [128, 129], F32R, kind="ExternalInput").ap()
    onesr_in = nc.dram_tensor("onesr", [1, 128], F32R, kind="ExternalInput").ap()
    y = nc.dram_tensor("y", [BL, D], F32, kind="ExternalOutput").ap()

    # internal-DRAM bounce buffers for row<->column reshapes
    qscr = nc.dram_tensor("qscr", [QH, 16, 128], F32R).ap()
    kscr = nc.dram_tensor("kscr", [16, 128], F32).ap()
    dscr = nc.dram_tensor("dscr", [8, 4, 128], F32).ap()

    NLP = BL // 256          # 8 column chunks for the x stream
    ND = D // 128            # 16 contraction chunks for the projections
    NS = S // 128            # 16 s chunks per batch
    NSC = CACHE // 128       # 8 cached s chunks

    with tile.TileContext(nc) as tc, ExitStack() as ctx:
        singles = ctx.enter_context(tc.tile_pool(name="singles", bufs=1))
        xtp = ctx.enter_context(tc.tile_pool(name="xtp", bufs=2))
        # rotating pool of [128, <=512] working tiles: exp chunks, squares,
        # bcast factors, output staging
        work = ctx.enter_context(tc.tile_pool(name="work", bufs=7))
        cachep = ctx.enter_context(tc.tile_pool(name="cachep", bufs=1))
        frp = ctx.enter_context(tc.tile_pool(name="frp", bufs=2))
        colp = ctx.enter_context(tc.tile_pool(name="colp", bufs=2))

        psA = ctx.enter_context(tc.tile_pool(name="psA", bufs=3, space="PSUM"))
        psO = ctx.enter_context(tc.tile_pool(name="psO", bufs=2, space="PSUM"))
        psD = ctx.enter_context(tc.tile_pool(name="psD", bufs=1, space="PSUM"))
        psF = ctx.enter_context(tc.tile_pool(name="psF", bufs=2, space="PSUM"))

        lowp = nc.allow_low_precision(reason="float32r rounding is intended")
        ctx.enter_context(lowp)

        consts = singles.tile([128, 129], F32R)
        nc.scalar.dma_start(out=consts, in_=consts_in)
        ident = consts[:, 0:128]
        ones_col = consts[:, 128:129]
        ones_row = singles.tile([1, 128], F32R)
        nc.scalar.dma_start(out=ones_row, in_=onesr_in)

        bias_q = singles.tile([1, 1], F32)
        nc.vector.memset(bias_q, HD * EPS)
        bias_k = singles.tile([1, 1], F32)
        nc.vector.memset(bias_k, EPS)

        wg_sb = singles.tile([H, QH], F32R)
        nc.scalar.dma_start(out=wg_sb, in_=wg)
        # split the weight load so the first projection matmuls start after
        # ~1/4 of it has landed
        wqkv_sb = singles.tile([128, ND, 4 * HD], F32R)
        wqkv_r = wqkv.rearrange("(k p) j -> p k j", p=128)
        for kq in range(4):
            nc.sync.dma_start(
                out=wqkv_sb[:, kq * 4 : kq * 4 + 4, :],
                in_=wqkv_r[:, kq * 4 : kq * 4 + 4, :],
            )
        wo_sb = singles.tile([128, QH, D], F32R)

        # persistent activations, feature-on-partition
        qkvt = singles.tile([128, 4, BL], F32R)       # jc: qh0, qh1, k, v
        otg = singles.tile([128, B, QH, 2, 512], F32R)  # raw attention out
        gcol = singles.tile([128, 16, QH], F32)       # gates, l-on-partition
        fcols = singles.tile([128, B, QH, 2, 4], F32)   # gate/den columns
        qs = [
            singles.tile([1, BL], F32R, tag=f"qs{i}", name=f"qs{i}")
            for i in range(QH)
        ]  # q rms scale rows (sqrt then reciprocal via bounce)
        kcol = singles.tile([128, 16], F32)          # k rms scale columns

        ksr = singles.tile([1, BL], F32)             # k sqrt staging row
        xg = singles.tile([H, BL], F32R)              # x[..., :16] for gates
        cache_tiles = {}

        def emit_prefetch():
            """Non-critical loads, queued after the first x tile so they
            don't delay the first projection matmul."""
            nc.scalar.dma_start(
                out=wo_sb, in_=wo.rearrange("(h p) m -> p h m", p=128)
            )
            nc.scalar.dma_start(out=xg, in_=xt[0:H, :])
            for b in range(B):
                ck_sb = cachep.tile(
                    [128, CACHE], F32R, tag=f"ck{b}", name=f"ck{b}"
                )
                nc.scalar.dma_start(out=ck_sb, in_=ckt[b])
                cv_sb = cachep.tile(
                    [128, NSC, HD], F32R, tag=f"cv{b}", name=f"cv{b}"
                )
                nc.scalar.dma_start(
                    out=cv_sb, in_=cv[b].rearrange("(i p) d -> p i d", p=128)
                )
                cache_tiles[b] = (ck_sb, cv_sb)

        # ---- phase 1: projections -------------------------------------
        def finish_half(half):
            """Reciprocal of the rms rows via DRAM bounce (1-lane DVE rows
            are ~6us each) + qT column normalize — per half so the second
            half overlaps remaining projection work."""
            rs = slice(half * 8, half * 8 + 8)
            row_sl = slice(half * 1024, half * 1024 + 1024)
            nc.scalar.dma_start(out=kscr[rs], in_=ksr[:, row_sl])
            nc.scalar.dma_start(
                out=kcol[:, half * 8 : half * 8 + 8],
                in_=kscr[rs].rearrange("c p -> p c"),
            )
            nc.vector.reciprocal(
                kcol[:, half * 8 : half * 8 + 8], kcol[:, half * 8 : half * 8 + 8]
            )
            for h in range(QH):
                nc.scalar.dma_start(out=qscr[h, rs], in_=qs[h][:, row_sl])
                qc = colp.tile([128, 8], F32R, tag="qcol", name=f"qc{h}_{half}")
                nc.scalar.dma_start(out=qc, in_=qscr[h, rs].rearrange("c p -> p c"))
                nc.vector.reciprocal(qc, qc)
                nc.scalar.dma_start(out=qscr[h, rs].rearrange("c p -> p c"), in_=qc)
                nc.scalar.dma_start(
                    out=qs[h][:, row_sl], in_=qscr[h, rs].flatten().unsqueeze(0)
                )
                for lc in range(2):
                    sl = slice(half * 1024 + lc * 512, half * 1024 + lc * 512 + 512)
                    bc = psF.tile([128, 512], F32, tag="psF", name="bc")
                    nc.tensor.matmul(
                        bc, ones_row, qs[h][:, sl], start=True, stop=True
                    )
                    nc.vector.tensor_mul(qkvt[:, h, sl], qkvt[:, h, sl], bc)

        xt_r = xt.rearrange("(k p) l -> p k l", p=128)
        for lc in range(NLP):
            sl = slice(lc * 256, lc * 256 + 256)
            xtile = xtp.tile([128, ND, 256], F32R, tag="xt")
            for kq in range(4):
                nc.sync.dma_start(
                    out=xtile[:, kq * 4 : kq * 4 + 4, :],
                    in_=xt_r[:, kq * 4 : kq * 4 + 4, sl],
                )
            if lc == 0:
                emit_prefetch()
            for jc in (3, 2, 0, 1):  # v and k first: unblocks attention prep
                pp = psA.tile([128, 256], F32, tag="psA")
                for kk in range(ND):
                    nc.tensor.matmul(
                        pp,
                        wqkv_sb[:, kk, jc * 128 : jc * 128 + 128],
                        xtile[:, kk, :],
                        start=(kk == 0),
                        stop=(kk == ND - 1),
                    )
                nc.vector.tensor_copy(qkvt[:, jc, sl], pp)
                if jc < 3:  # q0, q1, k need sum over HD of the square
                    sq = work.tile([128, 256], F32R, tag="work", name=f"sq{lc}_{jc}")
                    nc.vector.tensor_mul(sq, qkvt[:, jc, sl], qkvt[:, jc, sl])
                    ssq = psD.tile([1, 256], F32, tag="psD")
                    nc.tensor.matmul(ssq, ones_col, sq, start=True, stop=True)
                    # q: sqrt(ssq + HD*eps) so the reciprocal also folds in
                    # the 1/sqrt(HD) score scale; k: sqrt(ssq/HD + eps).
                    row = qs[jc] if jc < QH else ksr
                    scale, bias = (1.0, bias_q) if jc < QH else (1.0 / HD, bias_k)
                    nc.scalar.activation(
                        row[:, sl], ssq, AF.Sqrt, bias=bias[:], scale=scale
                    )
            if lc == 3:
                finish_half(0)
        finish_half(1)
        # gates in column form: [l-part, chunk, head]
        gps = psF.tile([128, 16, QH], F32, tag="psF", name="gps")
        for c in range(16):
            nc.tensor.matmul(
                gps[:, c, :],
                xg[:, c * 128 : c * 128 + 128],
                wg_sb,
                start=True,
                stop=True,
            )
        nc.scalar.activation(gcol, gps, AF.Sigmoid)

        # ---- phase 2: attention ---------------------------------------
        for b in range(B):
            boff = b * L
            ck_sb, cv_sb = cache_tiles[b]
            vnew = cachep.tile([128, NSC, HD], F32R, tag=f"vnew{b}", name=f"vn{b}")
            for i in range(NSC):
                tp = psF.tile([128, 128], F32R, tag="psF", name="tp")
                nc.tensor.transpose(
                    tp, qkvt[:, 3, boff + i * 128 : boff + i * 128 + 128], ident
                )
                nc.vector.tensor_copy(vnew[:, i, :], tp)

            for h in range(QH):
                for lc2 in range(2):
                    it = (b * QH + h) * 2 + lc2
                    off = boff + lc2 * 512
                    qsl = qkvt[:, h, off : off + 512]
                    den = psD.tile([1, 512], F32, tag="psD")
                    ot = psO.tile([128, 512], F32)
                    for sc in range(NS):
                        if sc < NSC:
                            kT = ck_sb[:, sc * 128 : sc * 128 + 128]
                            vx = cv_sb[:, sc, :]
                            kscale = 1.0
                        else:
                            j = boff + (sc - NSC) * 128
                            kT = qkvt[:, 2, j : j + 128]
                            vx = vnew[:, sc - NSC, :]
                            cglob = (sc - NSC) + 8 * b
                            kscale = kcol[:, cglob : cglob + 1]
                        sp = psA.tile([128, 512], F32, tag="psA")
                        nc.tensor.matmul(sp, kT, qsl, start=True, stop=True)
                        ex = work.tile([128, 512], F32R, tag="work", name=f"ex{sc}")
                        nc.scalar.activation(ex, sp, AF.Exp, scale=kscale)
                        nc.tensor.matmul(
                            den, ones_col, ex,
                            start=(sc == 0), stop=(sc == NS - 1),
                        )
                        nc.tensor.matmul(
                            ot, vx, ex,
                            start=(sc == 0), stop=(sc == NS - 1),
                        )
                    # evacuate raw attention out immediately so the PSUM
                    # accumulator recycles without waiting on the factor
                    # chain; gate/den applied in phase 3 as a per-partition
                    # scale
                    nc.vector.tensor_copy(otg[:, b, h, lc2, :], ot)
                    drow = frp.tile([1, 512], F32, tag="drow", name="drow")
                    nc.scalar.copy(drow, den)
                    nc.scalar.dma_start(out=dscr[it], in_=drow)
                    dcol = colp.tile([128, 4], F32, tag="dcol", name="dcol")
                    nc.scalar.dma_start(
                        out=dcol, in_=dscr[it].rearrange("c p -> p c")
                    )
                    nc.vector.reciprocal(dcol, dcol)
                    nc.vector.tensor_mul(
                        fcols[:, b, h, lc2, :],
                        dcol,
                        gcol[:, 8 * b + 4 * lc2 : 8 * b + 4 * lc2 + 4, h],
                    )

        # ---- phase 3: partial out-projection --------------------------
        for b in range(B):
            for lc2 in range(2):
                for li in range(4):
                    row0 = b * L + lc2 * 512 + li * 128
                    for mc in range(4):
                        yps = []
                        for h in range(QH):
                            yp = psA.tile(
                                [128, 512], F32, tag="psA", name=f"yp{h}"
                            )
                            nc.tensor.matmul(
                                yp,
                                otg[:, b, h, lc2, li * 128 : li * 128 + 128],
                                wo_sb[:, h, mc * 512 : mc * 512 + 512],
                                start=True,
                                stop=True,
                            )
                            yps.append(yp)
                        # ysb = f0[l]*yp0 + f1[l]*yp1  (f per-partition);
                        # step 1 on ACT, step 2 on DVE — phase 3 is
                        # evacuation-bound, so split it across engines
                        ysb = work.tile([128, 512], F32, tag="work", name="ysb")
                        nc.scalar.activation(
                            ysb,
                            yps[0],
                            AF.Identity,
                            scale=fcols[:, b, 0, lc2, li : li + 1],
                        )
                        nc.vector.scalar_tensor_tensor(
                            out=ysb,
                            in0=yps[1],
                            scalar=fcols[:, b, 1, lc2, li : li + 1],
                            in1=ysb,
                            op0=mybir.AluOpType.mult,
                            op1=mybir.AluOpType.add,
                        )
                        nc.sync.dma_start(
                            out=y[row0 : row0 + 128, mc * 512 : mc * 512 + 512],
                            in_=ysb,
                        )

    nc.compile()
    return nc


def _get_nc():
    global _CACHED_NC
    if _CACHED_NC is None:
        _CACHED_NC = _build_core_program()
    return _CACHED_NC


def make_in_maps(x, w_q, w_k, w_v, w_out, w_gate, cache_k, cache_v):
    xt = np.ascontiguousarray(x.reshape(BL, D).T)
    consts_np = np.concatenate(
        [np.eye(128, dtype=np.float32), np.ones((128, 1), np.float32)], axis=1
    )
    in_maps = []
    for c in range(NCORES):
        g = c // 2
        wq_c = w_q[c * JC : (c + 1) * JC]                      # [256, D]
        wk_c = w_k[g * HD : (g + 1) * HD]                      # [128, D]
        wv_c = w_v[g * HD : (g + 1) * HD]
        wqkv_c = np.ascontiguousarray(
            np.concatenate([wq_c, wk_c, wv_c], axis=0).T      # [D, 512]
        )
        wo_c = np.ascontiguousarray(w_out[:, c * JC : (c + 1) * JC].T)  # [256, D]
        wg_c = np.ascontiguousarray(w_gate[c * QH : (c + 1) * QH].T)    # [16, 2]
        ckt_c = np.ascontiguousarray(cache_k[:, g].transpose(0, 2, 1))  # [B,HD,CACHE]
        cv_c = np.ascontiguousarray(cache_v[:, g])                      # [B,CACHE,HD]
        in_maps.append(
            {
                "xt": xt,
                "wqkv": wqkv_c,
                "wo": wo_c,
                "wg": wg_c,
                "ckt": ckt_c,
                "cv": cv_c,
                "consts": consts_np,
                "onesr": np.ones((1, 128), np.float32),
            }
        )
    return in_maps


def kernel(x, w_q, w_k, w_v, w_out, w_gate, cache_k, cache_v, _run_kwargs=None):
    in_maps = make_in_maps(x, w_q, w_k, w_v, w_out, w_gate, cache_k, cache_v)
    nc = _get_nc()
    res = run_bass_kernel_spmd(
        nc, in_maps, core_ids=list(range(NCORES)), **(_run_kwargs or {})
    )
    acc = np.zeros((BL, D), dtype=np.float64)
    for c in range(NCORES):
        acc += res.results[c]["y"]
    out = acc.astype(np.float32).reshape(B, L, D)
    if _run_kwargs:
        kernel.last_results = res
    return out

